# revision 11
# baseline (speedup 1.0000x reference)
"""DenseOSTL one-step kernel for Trainium2 (8 NeuronCores, data-parallel over batch).

Math (per sample s):
    I       = x @ W + b
    v       = BETA*u0 + I
    spk     = (v - THRESH > 0)
    sg      = 1 / (1 + |v - THRESH|)^2
    u_new   = v - THRESH*spk
    one_m   = 1 - THRESH*sg
    E_b_new = one_m * (BETA*E_b + 1)                      [since du_du = BETA*one_m]
    E_W_new[i,j] = one_m[j] * (BETA*E_W[i,j] + x[i])      [rank-1 factored form]

The E_W stream (256 MB in + 256 MB out across the batch) dominates; everything
else is noise. Each core owns 8 samples: 32 MB in + 32 MB out.
"""

import json
import os
import sys

for _p in ("/opt/trn_rl_repo", "/root/.axon_site/_ro/trn_rl_repo"):
    if os.path.isdir(_p) and _p not in sys.path:
        sys.path.append(_p)

from contextlib import ExitStack

import numpy as np

import concourse.bass as bass
import concourse.tile as tile
from concourse import mybir
from concourse.bass_utils import run_bass_kernel_spmd

B, DIN, DOUT = 64, 1024, 1024
NCORES = 8
S = B // NCORES            # samples per core
P = 128                    # SBUF partitions
KCH = DIN // P             # k-chunks of Din
CH_PER = 4                 # Din-chunks per DMA transfer (4 * 512KB = 2MB)
BETA, THRESH = 0.9, 1.0

F32 = mybir.dt.float32
AF = mybir.ActivationFunctionType
ALU = mybir.AluOpType


def build_program(repeat: int = 1) -> bass.Bass:
    """repeat>1 re-runs the E_W stream phase (idempotent: same outputs) so
    wall-clock deltas between repeat variants isolate on-device kernel time."""
    nc = bass.Bass()

    x_d = nc.dram_tensor("x", [S, DIN], F32, kind="ExternalInput")
    w_d = nc.dram_tensor("W", [DIN, DOUT], F32, kind="ExternalInput")
    b_d = nc.dram_tensor("b", [DOUT], F32, kind="ExternalInput")
    u0_d = nc.dram_tensor("u0", [S, DOUT], F32, kind="ExternalInput")
    ew_d = nc.dram_tensor("E_W", [S, DIN, DOUT], F32, kind="ExternalInput")
    eb_d = nc.dram_tensor("E_b", [S, DOUT], F32, kind="ExternalInput")
    eye_d = nc.dram_tensor("eye8", [S, S], F32, kind="ExternalInput")

    spk_d = nc.dram_tensor("spk", [S, DOUT], F32, kind="ExternalOutput")
    un_d = nc.dram_tensor("u_new", [S, DOUT], F32, kind="ExternalOutput")
    ewn_d = nc.dram_tensor("E_W_new", [S, DIN, DOUT], F32, kind="ExternalOutput")
    ebn_d = nc.dram_tensor("E_b_new", [S, DOUT], F32, kind="ExternalOutput")

    # [s, i, j] -> [s, p, c, j] with i = c*128 + p: per-partition rows stay
    # contiguous 4KB runs in DRAM, so DMA descriptors are well-shaped.
    ew_r = ew_d.rearrange("s (c p) n -> s p c n", p=P)
    ewn_r = ewn_d.rearrange("s (c p) n -> s p c n", p=P)

    with tile.TileContext(nc) as tc, ExitStack() as ctx:
        const = ctx.enter_context(tc.tile_pool(name="const", bufs=1))
        wpool = ctx.enter_context(tc.tile_pool(name="wpool", bufs=1))
        small = ctx.enter_context(tc.tile_pool(name="small", bufs=1))
        bcpool = ctx.enter_context(tc.tile_pool(name="bcpool", bufs=3))
        stream = ctx.enter_context(tc.tile_pool(name="stream", bufs=4))
        ps_t = ctx.enter_context(tc.tile_pool(name="ps_t", bufs=2, space="PSUM"))
        ps_i = ctx.enter_context(tc.tile_pool(name="ps_i", bufs=1, space="PSUM"))
        ps_bc = ctx.enter_context(tc.tile_pool(name="ps_bc", bufs=2, space="PSUM"))

        # --- auxiliary loads (SWDGE ring; keeps the HWDGE rings for E_W) ---
        x_sb = small.tile([S, DIN], F32)
        nc.gpsimd.dma_start(out=x_sb[:], in_=x_d[:])
        eye_sb = const.tile([S, S], F32)
        nc.gpsimd.dma_start(out=eye_sb[:], in_=eye_d[:])
        w_sb = wpool.tile([P, KCH, DOUT], F32)
        nc.gpsimd.dma_start(out=w_sb[:], in_=w_d.rearrange("(c p) n -> p c n", p=P))
        b_sb = small.tile([1, DOUT], F32)
        nc.gpsimd.dma_start(out=b_sb[:], in_=b_d.rearrange("(o n) -> o n", o=1))
        u0_sb = small.tile([S, DOUT], F32)
        nc.gpsimd.dma_start(out=u0_sb[:], in_=u0_d[:])
        eb_sb = small.tile([S, DOUT], F32)
        nc.gpsimd.dma_start(out=eb_sb[:], in_=eb_d[:])

        ones_m = const.tile([1, P], F32)
        nc.vector.memset(ones_m[:], 1.0)

        # --- xT[p, k, s] = x[s, k*128+p] via PE transpose ---
        xT = small.tile([P, KCH, S], F32)
        for k in range(KCH):
            pt = ps_t.tile([P, S], F32)
            nc.tensor.transpose(pt[:], x_sb[:, k * P:(k + 1) * P], eye_sb[:])
            nc.scalar.copy(xT[:, k, :], pt[:])

        # --- I = x @ W + b  -> PSUM [S, DOUT] ---
        ps_I = ps_i.tile([S, DOUT], F32)
        for n in range(2):
            nsl = slice(n * 512, (n + 1) * 512)
            for k in range(KCH):
                nc.tensor.matmul(ps_I[:, nsl], xT[:, k, :], w_sb[:, k, nsl],
                                 start=(k == 0), stop=False)
            # bias via ones-row: I += 1 (x) b
            nc.tensor.matmul(ps_I[:, nsl], ones_m[:, 0:S], b_sb[:, nsl],
                             start=False, stop=True)

        # --- small elementwise chain (all [S, DOUT]) ---
        v = small.tile([S, DOUT], F32)
        nc.vector.tensor_scalar(v[:], u0_sb[:], BETA, None, ALU.mult)
        nc.vector.tensor_tensor(v[:], v[:], ps_I[:], ALU.add)

        spk = small.tile([S, DOUT], F32)
        nc.vector.tensor_scalar(spk[:], v[:], THRESH, None, ALU.is_gt)
        un = small.tile([S, DOUT], F32)
        nc.vector.tensor_tensor(un[:], v[:], spk[:], ALU.subtract)
        nc.gpsimd.dma_start(out=spk_d[:], in_=spk[:])
        nc.gpsimd.dma_start(out=un_d[:], in_=un[:])

        neg_th = small.tile([S, 1], F32)
        nc.vector.memset(neg_th[:], -THRESH)
        w1 = small.tile([S, DOUT], F32)
        nc.scalar.activation(w1[:], v[:], AF.Abs, bias=neg_th[:], scale=1.0)  # |v-1|
        nc.vector.tensor_scalar(w1[:], w1[:], 1.0, None, ALU.add)   # 1 + |vt|
        sqw = small.tile([S, DOUT], F32)
        nc.vector.tensor_tensor(sqw[:], w1[:], w1[:], ALU.mult)     # (1+|vt|)^2
        sg = small.tile([S, DOUT], F32)
        nc.vector.reciprocal(sg[:], sqw[:])
        one_m = small.tile([S, DOUT], F32)
        nc.vector.tensor_scalar(one_m[:], sg[:], -THRESH, 1.0, ALU.mult, ALU.add)

        ebn = small.tile([S, DOUT], F32)
        nc.vector.tensor_scalar(ebn[:], eb_sb[:], BETA, 1.0, ALU.mult, ALU.add)
        nc.vector.tensor_tensor(ebn[:], ebn[:], one_m[:], ALU.mult)
        nc.gpsimd.dma_start(out=ebn_d[:], in_=ebn[:])

        # PE matmul rhs must start at partition 0 -> flatten one_m rows onto
        # partition 0 with a small SBUF->SBUF DMA.
        onem_flat = small.tile([1, S, DOUT], F32)
        nc.gpsimd.dma_start(out=onem_flat[:], in_=one_m[:])

        # --- E_W stream: per sample, broadcast one_m then 2MB-granular tiles ---
        n_h = KCH // CH_PER
        for _rep, s in ((r, s) for r in range(repeat) for s in range(S)):
            bc_ps = ps_bc.tile([P, DOUT], F32)
            for n in range(2):
                nsl = slice(n * 512, (n + 1) * 512)
                nc.tensor.matmul(bc_ps[:, nsl], ones_m[:], onem_flat[:, s, nsl],
                                 start=True, stop=True)
            bc_sb = bcpool.tile([P, DOUT], F32)
            nc.scalar.copy(bc_sb[:], bc_ps[:])

            for h in range(n_h):
                csl = slice(h * CH_PER, (h + 1) * CH_PER)
                et = stream.tile([P, CH_PER, DOUT], F32)
                nc.sync.dma_start(out=et[:], in_=ew_r[s][:, csl, :])
                for c in range(CH_PER):
                    k = h * CH_PER + c
                    nc.scalar.activation(et[:, c, :], et[:, c, :], AF.Identity,
                                         bias=xT[:, k, s:s + 1], scale=BETA)
                    nc.vector.tensor_tensor(et[:, c, :], et[:, c, :], bc_sb[:],
                                            ALU.mult)
                nc.scalar.dma_start(out=ewn_r[s][:, csl, :], in_=et[:])

    _split_multiwaits(nc)
    return nc


def _split_multiwaits(nc: bass.Bass) -> None:
    """This toolchain's walrus accepts at most ONE sync wait per instruction
    (\"Too many sync wait commands\"). Tile attaches all of an instruction's
    waits to its sync_info, so split the overflow onto same-engine NoOps
    inserted immediately before — semantically identical. Patch the JSON the
    compiler consumes; the in-memory module (used by CoreSim) is untouched."""
    js = json.loads(nc.to_json_bytes())
    for fn in js["functions"]:
        for blk in fn["blocks"]:
            newinsts = []
            for inst in blk["instructions"]:
                si = inst.get("sync_info")
                waits = (si or {}).get("on_wait") or []
                if len(waits) > 1:
                    for j, w in enumerate(waits[:-1]):
                        newinsts.append({
                            "name": f"{inst['name']}-wfix{j}",
                            "engine": inst["engine"],
                            "opcode": "NoOp",
                            "ins": [],
                            "outs": [],
                            "sync_info": {"on_wait": [w], "on_update": []},
                        })
                    si["on_wait"] = [waits[-1]]
                newinsts.append(inst)
            blk["instructions"] = newinsts
    blob = json.dumps(js).encode()
    nc.to_json_bytes = lambda: blob


_PROGS = {}


def _get_program(repeat: int = 1) -> bass.Bass:
    if repeat not in _PROGS:
        _PROGS[repeat] = build_program(repeat)
    return _PROGS[repeat]


def make_in_maps(x, W, b, u0, E_W, E_b):
    eye = np.eye(S, dtype=np.float32)
    maps = []
    for c in range(NCORES):
        sl = slice(c * S, (c + 1) * S)
        maps.append({
            "x": np.ascontiguousarray(x[sl]),
            "W": np.ascontiguousarray(W),
            "b": np.ascontiguousarray(b),
            "u0": np.ascontiguousarray(u0[sl]),
            "E_W": np.ascontiguousarray(E_W[sl]),
            "E_b": np.ascontiguousarray(E_b[sl]),
            "eye8": eye,
        })
    return maps


def gather_outputs(results):
    spk = np.concatenate([r["spk"] for r in results], axis=0)
    u_new = np.concatenate([r["u_new"] for r in results], axis=0)
    ew_new = np.concatenate([r["E_W_new"] for r in results], axis=0)
    eb_new = np.concatenate([r["E_b_new"] for r in results], axis=0)
    return spk, u_new, ew_new, eb_new


def kernel(x, W, b, u0, E_W, E_b, _trace=False, _results_out=None):
    x = np.asarray(x, np.float32)
    W = np.asarray(W, np.float32)
    b = np.asarray(b, np.float32)
    u0 = np.asarray(u0, np.float32)
    E_W = np.asarray(E_W, np.float32)
    E_b = np.asarray(E_b, np.float32)

    nc = _get_program()
    res = run_bass_kernel_spmd(nc, make_in_maps(x, W, b, u0, E_W, E_b),
                               core_ids=list(range(NCORES)), trace=_trace)
    if _results_out is not None:
        _results_out.append(res)
    return gather_outputs(res.results)
